# revision 1
# baseline (speedup 1.0000x reference)
"""GraphSAGE 2-layer kernel for 8 Trainium2 NeuronCores (SPMD).

Strategy:
- Nodes sorted by in-degree, padded to NPOS = 8*128*NBLK positions, dealt
  round-robin to cores in 128-lane blocks (run j = 8 cores' block j, all with
  near-equal degree). One reserved always-zero lane per core (last block,
  lane 127) serves as the gather pad target.
- Layer-1 aggregation: host expands x[src] into the canonical slot layout
  [128 lanes x cols x 64] bf16 (edges of dst lane p in column k); the device
  segment-sum is a plain PSUM accumulation of identity matmuls.
- h (f32) is AllGathered across cores; layer-2 aggregation gathers h rows
  with gpsimd.dma_gather (int16 indices) from 4 windows of 2 core-shards
  each (25088 rows < 32767), pads point at the reserved zero rows.
- Dense part per block: Y = [mean | x] @ [Wl; Wr] via one matmul with
  lhsT = [mean^T; x^T]; mean^T from a PE transpose, x^T host-provided
  (layer 1) or PE-transposed from own h shard (layer 2). L2-normalize via
  ACT square+accum, sqrt(+1e-24), DVE reciprocal, fused scale(+relu).
"""
import numpy as np
import ml_dtypes

import concourse.bass as bass
import concourse.bacc as bacc
import concourse.tile as tile
from concourse import mybir
from concourse import bass_utils

NCORES = 8
LANES = 128
BPG = 8           # blocks per psum group (psum free = BPG*64 = 512)
L1_CHUNK_COLS = 96
L2_CHUNK_IDX = 12288   # max gather indices per instruction (HW cap ~15k)
F_IN, F_HID, F_OUT = 64, 64, 32
BF16 = ml_dtypes.bfloat16


def _wrap_idx(flat_idx):
    """flat [n] int16 -> [128, n/16] wrapped in 16 partitions, replicated x8."""
    n = flat_idx.shape[0]
    arr = flat_idx.reshape(n // 16, 16).T
    return np.tile(arr, (8, 1)).astype(np.int16)


def _preprocess(x, edge_index, N):
    src = np.asarray(edge_index[0], dtype=np.int64)
    dst = np.asarray(edge_index[1], dtype=np.int64)
    E = src.shape[0]

    nblk = int(np.ceil((N + NCORES) / (NCORES * LANES)))
    npos = NCORES * LANES * nblk
    npc = LANES * nblk
    winrows = 2 * npc
    nwin = NCORES // 2
    ngrp = int(np.ceil(nblk / BPG))
    nb_g = [min(BPG, nblk - g * BPG) for g in range(ngrp)]

    deg = np.bincount(dst, minlength=N).astype(np.int64)

    # position space: all (run j, core c, lane p); reserved = (nblk-1, c, 127)
    res_pos = (nblk - 1) * NCORES * LANES + np.arange(NCORES) * LANES + (LANES - 1)
    avail = np.ones(npos, dtype=bool)
    avail[res_pos] = False
    avail_pos = np.flatnonzero(avail)
    nfill = npos - NCORES - N
    order = np.argsort(deg, kind="stable")
    pos2node = np.full(npos, -1, dtype=np.int64)
    pos2node[avail_pos[nfill:]] = order

    ii = np.arange(npos)
    pos_c = (ii % (NCORES * LANES)) // LANES
    pos_j = ii // (NCORES * LANES)
    pos_p = ii % LANES
    pos_row = pos_c * npc + pos_j * 128 + pos_p
    node2row = np.empty(N, dtype=np.int64)
    real = pos2node >= 0
    node2row[pos2node[real]] = pos_row[real]

    # per-run degree maxima
    degpos = np.where(real, deg[np.clip(pos2node, 0, None)], 0)
    run_deg = degpos.reshape(nblk, NCORES * LANES).max(axis=1)
    d1_g = [max(1, int(run_deg[g * BPG:g * BPG + nb_g[g]].max())) for g in range(ngrp)]

    # CSR by dst
    eord = np.argsort(dst, kind="stable")
    s_by_dst = src[eord]
    indptr = np.zeros(N + 1, dtype=np.int64)
    indptr[1:] = np.cumsum(deg)

    # ---- layer-2 window structure ----
    srow = node2row[src]
    w_e = srow // winrows
    cnt = np.zeros((N, nwin), dtype=np.int64)
    np.add.at(cnt, (dst, w_e), 1)
    cnt_pos = np.where(real[:, None], cnt[np.clip(pos2node, 0, None)], 0)
    # run x window maxima -> group x window maxima
    d2_run = cnt_pos.reshape(nblk, NCORES * LANES, nwin).max(axis=1)
    d2_g = np.zeros((ngrp, nwin), dtype=np.int64)
    for g in range(ngrp):
        d2_g[g] = d2_run[g * BPG:g * BPG + nb_g[g]].max(axis=0)
        if d2_g[g].sum() == 0:
            d2_g[g][0] = 1

    ekey = dst * nwin + w_e
    eord2 = np.argsort(ekey, kind="stable")
    rloc_sorted = (srow % winrows)[eord2]
    # indptr2[n, w] = start of node n's window-w edges in rloc_sorted
    indptr2 = indptr[:-1, None] + np.concatenate(
        [np.zeros((N, 1), dtype=np.int64), np.cumsum(cnt, axis=1)[:, :-1]], axis=1)

    ZPAD = npc - 1  # local row of the reserved zero lane in a window's first shard

    # per-core host arrays
    xbf = np.asarray(x, dtype=np.float32).astype(BF16)
    xf = np.asarray(x, dtype=np.float32)

    # node id at (c, j, p)
    node_cjp = np.full((NCORES, nblk, LANES), -1, dtype=np.int64)
    node_cjp[pos_c[real], pos_j[real], pos_p[real]] = pos2node[real]

    deg_cjp = np.where(node_cjp >= 0, deg[np.clip(node_cjp, 0, None)], 0)
    ip_cjp = np.where(node_cjp >= 0, indptr[np.clip(node_cjp, 0, None)], 0)

    # ---- L1 slots expansion + schedule ----
    tot1 = sum(d1_g[g] * nb_g[g] for g in range(ngrp))
    slots1 = [np.zeros((128, tot1, F_IN), dtype=BF16) for _ in range(NCORES)]
    l1_sched = []  # per group: (col_offset, d1, nb)
    cofs = 0
    for g in range(ngrp):
        d1, nb = d1_g[g], nb_g[g]
        l1_sched.append((cofs, d1, nb))
        for b in range(nb):
            j = g * BPG + b
            for c in range(NCORES):
                nodes = node_cjp[c, j]
                db = deg_cjp[c, j]
                base = ip_cjp[c, j][:, None] + np.arange(d1)[None, :]
                valid = np.arange(d1)[None, :] < db[:, None]
                sidx = np.where(valid, s_by_dst[np.clip(base, 0, E - 1)], 0)
                vals = np.where(valid[:, :, None], xbf[sidx], BF16(0))
                # columns k -> global col cofs + k*nb + b
                slots1[c][:, cofs + b + np.arange(d1) * nb, :] = vals
        cofs += d1 * nb
    assert cofs == tot1

    # ---- L2 idx arrays + schedule ----
    idx_parts = [[] for _ in range(NCORES)]
    l2_sched = []  # per group: list of (w, ncols_chunk, nk_chunk)
    for g in range(ngrp):
        nb = nb_g[g]
        wlist = []
        for w in range(nwin):
            D = int(d2_g[g][w])
            if D == 0:
                continue
            # per-core idx matrix [128, D*nb]
            mats = []
            for c in range(NCORES):
                mat = np.full((128, D, nb), ZPAD, dtype=np.int64)
                for b in range(nb):
                    j = g * BPG + b
                    nodes = node_cjp[c, j]
                    cw = np.where(nodes >= 0, cnt[np.clip(nodes, 0, None), w], 0)
                    ip = np.where(nodes >= 0, indptr2[np.clip(nodes, 0, None), w], 0)
                    base = ip[:, None] + np.arange(D)[None, :]
                    valid = np.arange(D)[None, :] < cw[:, None]
                    mat[:, :, b] = np.where(
                        valid, rloc_sorted[np.clip(base, 0, E - 1)], ZPAD)
                mats.append(mat.reshape(128, D * nb))
            # chunk k's
            kmax_per_chunk = max(1, L2_CHUNK_IDX // (128 * nb))
            k0 = 0
            chunks = []
            while k0 < D:
                nk = min(kmax_per_chunk, D - k0)
                ncols = nk * nb
                for c in range(NCORES):
                    sub = mats[c][:, k0 * nb:(k0 + nk) * nb]  # [128, ncols]
                    flat = sub.T.reshape(-1)  # slot s = col*128 + p
                    idx_parts[c].append(_wrap_idx(flat.astype(np.int16)))
                chunks.append((ncols, nk))
                k0 += nk
            wlist.append((w, chunks))
        l2_sched.append(wlist)

    idx2 = [np.concatenate(parts, axis=1) for parts in idx_parts]

    # ---- dense inputs ----
    xT = np.zeros((NCORES, nblk, F_IN, 128), dtype=np.float32)
    invc = np.zeros((NCORES, 128, nblk), dtype=np.float32)
    for c in range(NCORES):
        nodes = node_cjp[c]  # [nblk, 128]
        ok = nodes >= 0
        xv = np.where(ok[:, :, None], xf[np.clip(nodes, 0, None)], 0.0)
        xT[c] = xv.transpose(0, 2, 1)
        invc[c] = np.where(ok, 1.0 / np.maximum(deg_cjp[c], 1), 0.0).T

    meta = dict(nblk=nblk, npos=npos, npc=npc, winrows=winrows, ngrp=ngrp,
                nb_g=nb_g, d1_g=d1_g, l1_sched=l1_sched, l2_sched=l2_sched,
                tot1=tot1, idx2_cols=idx2[0].shape[1], node2row=node2row)
    per_core = dict(slots1=[s.reshape(128, tot1 * F_IN) for s in slots1],
                    idx2=idx2, xT=xT, invc=invc)
    return meta, per_core


def _build(meta, b1_nonzero, b2_nonzero):
    nblk, npc, npos = meta["nblk"], meta["npc"], meta["npos"]
    winrows, ngrp = meta["winrows"], meta["ngrp"]
    nb_g, l1_sched, l2_sched = meta["nb_g"], meta["l1_sched"], meta["l2_sched"]
    tot1, idx2_cols = meta["tot1"], meta["idx2_cols"]

    nc = bacc.Bacc("TRN2", target_bir_lowering=False, debug=False,
                   num_devices=NCORES)
    slots1 = nc.dram_tensor("slots1", [128, tot1 * F_IN], mybir.dt.bfloat16,
                            kind="ExternalInput")
    idx2 = nc.dram_tensor("idx2", [128, idx2_cols], mybir.dt.int16,
                          kind="ExternalInput")
    xT = nc.dram_tensor("xT", [nblk, F_IN, 128], mybir.dt.float32,
                        kind="ExternalInput")
    invc_d = nc.dram_tensor("invc", [128, nblk], mybir.dt.float32,
                            kind="ExternalInput")
    w1s = nc.dram_tensor("w1s", [128, F_HID], mybir.dt.float32,
                         kind="ExternalInput")
    w2s = nc.dram_tensor("w2s", [128, F_OUT], mybir.dt.float32,
                         kind="ExternalInput")
    b1t = nc.dram_tensor("b1t", [128, F_HID], mybir.dt.float32,
                         kind="ExternalInput")
    b2t = nc.dram_tensor("b2t", [128, F_OUT], mybir.dt.float32,
                         kind="ExternalInput")
    identf = nc.dram_tensor("identf", [128, 128], mybir.dt.float32,
                            kind="ExternalInput")
    identb = nc.dram_tensor("identb", [128, 128], mybir.dt.bfloat16,
                            kind="ExternalInput")
    out_d = nc.dram_tensor("out", [npc, F_OUT], mybir.dt.float32,
                           kind="ExternalOutput")

    with tile.TileContext(nc) as tc:
        with (
            tc.tile_pool(name="const", bufs=1) as cp,
            tc.tile_pool(name="slots", bufs=3) as sp,
            tc.tile_pool(name="gath", bufs=3) as gp,
            tc.tile_pool(name="idxp", bufs=3) as ixp,
            tc.tile_pool(name="blk", bufs=3) as bp,
            tc.tile_pool(name="psA", bufs=2, space="PSUM") as psA,
            tc.tile_pool(name="psT", bufs=2, space="PSUM") as psT,
            tc.tile_pool(name="psD", bufs=2, space="PSUM") as psD,
            tc.tile_pool(name="dram", bufs=1, space="DRAM") as dp,
        ):
            idf = cp.tile([128, 128], mybir.dt.float32, tag="idf")
            nc.sync.dma_start(idf[:], identf[:])
            idb = cp.tile([128, 128], mybir.dt.bfloat16, tag="idb")
            nc.sync.dma_start(idb[:], identb[:])
            w1 = cp.tile([128, F_HID], mybir.dt.float32, tag="w1")
            nc.sync.dma_start(w1[:], w1s[:])
            w2 = cp.tile([128, F_OUT], mybir.dt.float32, tag="w2")
            nc.sync.dma_start(w2[:], w2s[:])
            bt1 = cp.tile([128, F_HID], mybir.dt.float32, tag="bt1")
            nc.sync.dma_start(bt1[:], b1t[:])
            bt2 = cp.tile([128, F_OUT], mybir.dt.float32, tag="bt2")
            nc.sync.dma_start(bt2[:], b2t[:])
            icn = cp.tile([128, nblk], mybir.dt.float32, tag="icn")
            nc.sync.dma_start(icn[:], invc_d[:])
            zt = cp.tile([128, F_HID], mybir.dt.float32, tag="zt")
            nc.vector.memset(zt[:], 0.0)
            epst = cp.tile([128, 1], mybir.dt.float32, tag="epst")
            nc.vector.memset(epst[:], 1e-24)

            hshard = dp.tile([npc, F_HID], mybir.dt.float32)
            hgat = dp.tile([npos, F_HID], mybir.dt.float32)

            def dense_block(j, meanT_ps, actsT, wtile, btile, fdim, relu, outdst):
                """mean^T already in meanT_ps [64,128]; actsT[64:128] filled."""
                nc.vector.tensor_copy(actsT[0:64, :], meanT_ps[:])
                pd = psD.tile([128, fdim], mybir.dt.float32, space="PSUM", tag="pd")
                nc.tensor.matmul(out=pd[:], lhsT=actsT[:], rhs=wtile[:],
                                 start=True, stop=True)
                y = bp.tile([128, fdim], mybir.dt.float32, tag="y")
                nc.vector.tensor_tensor(out=y[:], in0=pd[:], in1=btile[:],
                                        op=mybir.AluOpType.add)
                sq = bp.tile([128, fdim], mybir.dt.float32, tag="sq")
                ss = bp.tile([128, 1], mybir.dt.float32, tag="ss")
                nc.scalar.activation(out=sq[:], in_=y[:],
                                     func=mybir.ActivationFunctionType.Square,
                                     accum_out=ss[:])
                s = bp.tile([128, 1], mybir.dt.float32, tag="s")
                nc.scalar.activation(out=s[:], in_=ss[:],
                                     func=mybir.ActivationFunctionType.Sqrt,
                                     bias=epst[:])
                rv = bp.tile([128, 1], mybir.dt.float32, tag="rv")
                nc.vector.reciprocal(rv[:], s[:])
                o = bp.tile([128, fdim], mybir.dt.float32, tag="o")
                if relu:
                    nc.vector.tensor_scalar(out=o[:], in0=y[:], scalar1=rv[:],
                                            scalar2=0.0,
                                            op0=mybir.AluOpType.mult,
                                            op1=mybir.AluOpType.max)
                else:
                    nc.vector.tensor_scalar_mul(o[:], y[:], rv[:])
                nc.sync.dma_start(outdst, o[:])

            # ---------------- layer 1 ----------------
            for g in range(ngrp):
                cofs, d1, nb = l1_sched[g]
                pa = psA.tile([128, 512], mybir.dt.float32, space="PSUM", tag="pa")
                k0 = 0
                first = True
                while k0 < d1:
                    nk = min(max(1, L1_CHUNK_COLS // nb), d1 - k0)
                    ncols = nk * nb
                    st = sp.tile([128, L1_CHUNK_COLS * F_IN], mybir.dt.bfloat16,
                                 tag="st")
                    nc.sync.dma_start(
                        st[:, :ncols * F_IN],
                        slots1[:, (cofs + k0 * nb) * F_IN:
                               (cofs + (k0 + nk) * nb) * F_IN])
                    for k in range(nk):
                        last = (k0 + k == d1 - 1)
                        nc.tensor.matmul(
                            out=pa[:, :nb * F_IN],
                            lhsT=idb[:],
                            rhs=st[:, k * nb * F_IN:(k + 1) * nb * F_IN],
                            start=first, stop=last)
                        first = False
                    k0 += nk
                for b in range(nb):
                    j = g * BPG + b
                    mean = bp.tile([128, F_IN], mybir.dt.float32, tag="mean")
                    nc.vector.tensor_scalar_mul(
                        mean[:], pa[:, b * F_IN:(b + 1) * F_IN],
                        icn[:, j:j + 1])
                    mT = psT.tile([64, 128], mybir.dt.float32, space="PSUM",
                                  tag="mT")
                    nc.tensor.transpose(out=mT[:], in_=mean[:], identity=idf[:])
                    actsT = bp.tile([128, 128], mybir.dt.float32, tag="actsT")
                    nc.sync.dma_start(actsT[64:128, :], xT[j, :, :])
                    dense_block(j, mT, actsT, w1, bt1, F_HID, True,
                                hshard[j * 128:(j + 1) * 128, :])
            # reserved lane must be exactly zero for use as gather pad target
            nc.sync.dma_start(hshard[npc - 1:npc, :], zt[0:1, :])

            # ---------------- all-gather ----------------
            nc.gpsimd.collective_compute(
                "AllGather", mybir.AluOpType.bypass,
                replica_groups=[list(range(NCORES))],
                ins=[hshard[:]], outs=[hgat[:]])

            # ---------------- layer 2 ----------------
            iofs = 0
            for g in range(ngrp):
                nb = nb_g[g]
                pa = psA.tile([128, 512], mybir.dt.float32, space="PSUM", tag="pa")
                first = True
                lastlist = [(wi, ci) for wi, (w, chunks) in enumerate(l2_sched[g])
                            for ci in range(len(chunks))]
                for wi, (w, chunks) in enumerate(l2_sched[g]):
                    for ci, (ncols, nk) in enumerate(chunks):
                        nidx = 128 * ncols
                        it = ixp.tile([128, L2_CHUNK_IDX // 16], mybir.dt.int16,
                                      tag="it")
                        nc.sync.dma_start(it[:, :nidx // 16],
                                          idx2[:, iofs:iofs + nidx // 16])
                        iofs += nidx // 16
                        gt = gp.tile([128, L2_CHUNK_IDX // 128 * F_HID],
                                     mybir.dt.float32, tag="gt")
                        gt3 = gt[:, :ncols * F_HID].rearrange(
                            "p (c f) -> p c f", c=ncols)
                        nc.gpsimd.dma_gather(
                            out_ap=gt3,
                            in_ap=hgat[w * winrows:(w + 1) * winrows, :],
                            idxs_ap=it[:, :nidx // 16],
                            num_idxs=nidx,
                            num_idxs_reg=nidx,
                            elem_size=F_HID,
                            single_packet=False)
                        for k in range(nk):
                            last = (wi, ci) == lastlist[-1] and k == nk - 1
                            nc.tensor.matmul(
                                out=pa[:, :nb * F_HID],
                                lhsT=idf[:],
                                rhs=gt[:, k * nb * F_HID:(k + 1) * nb * F_HID],
                                start=first, stop=last)
                            first = False
                for b in range(nb):
                    j = g * BPG + b
                    mean = bp.tile([128, F_HID], mybir.dt.float32, tag="mean")
                    nc.vector.tensor_scalar_mul(
                        mean[:], pa[:, b * F_HID:(b + 1) * F_HID],
                        icn[:, j:j + 1])
                    mT = psT.tile([64, 128], mybir.dt.float32, space="PSUM",
                                  tag="mT")
                    nc.tensor.transpose(out=mT[:], in_=mean[:], identity=idf[:])
                    hr = bp.tile([128, F_HID], mybir.dt.float32, tag="hr")
                    nc.sync.dma_start(hr[:], hshard[j * 128:(j + 1) * 128, :])
                    hT = psT.tile([64, 128], mybir.dt.float32, space="PSUM",
                                  tag="hT")
                    nc.tensor.transpose(out=hT[:], in_=hr[:], identity=idf[:])
                    actsT = bp.tile([128, 128], mybir.dt.float32, tag="actsT")
                    nc.scalar.copy(actsT[64:128, :], hT[:])
                    dense_block(j, mT, actsT, w2, bt2, F_OUT, False,
                                out_d[j * 128:(j + 1) * 128, :])
    nc.compile()
    return nc


def kernel(x, edge_index, W1l, b1, W1r, W2l, b2, W2r):
    x = np.asarray(x, dtype=np.float32)
    N = x.shape[0]
    meta, per_core = _preprocess(x, edge_index, N)

    w1s = np.concatenate([np.asarray(W1l, np.float32),
                          np.asarray(W1r, np.float32)], axis=0)
    w2s = np.concatenate([np.asarray(W2l, np.float32),
                          np.asarray(W2r, np.float32)], axis=0)
    b1t = np.tile(np.asarray(b1, np.float32)[None, :], (128, 1))
    b2t = np.tile(np.asarray(b2, np.float32)[None, :], (128, 1))
    identf = np.eye(128, dtype=np.float32)
    identb = identf.astype(BF16)

    nc = _build(meta, bool(np.any(b1)), bool(np.any(b2)))

    in_maps = []
    for c in range(NCORES):
        in_maps.append(dict(
            slots1=per_core["slots1"][c],
            idx2=per_core["idx2"][c],
            xT=per_core["xT"][c],
            invc=per_core["invc"][c],
            w1s=w1s, w2s=w2s, b1t=b1t, b2t=b2t,
            identf=identf, identb=identb,
        ))
    res = bass_utils.run_bass_kernel_spmd(nc, in_maps, core_ids=list(range(NCORES)))
    outs = np.concatenate([res.results[c]["out"] for c in range(NCORES)], axis=0)
    full = outs[meta["node2row"]]
    return full.astype(np.float32)


if __name__ == "__main__":
    rng = np.random.default_rng(0)
    N, E = 100000, 1000000
    x = rng.standard_normal((N, 64), dtype=np.float32)
    ei = rng.integers(0, N, size=(2, E)).astype(np.int64)
    out = kernel(x=x, edge_index=ei,
                 W1l=rng.standard_normal((64, 64), dtype=np.float32) / 8,
                 b1=np.zeros(64, np.float32),
                 W1r=rng.standard_normal((64, 64), dtype=np.float32) / 8,
                 W2l=rng.standard_normal((64, 32), dtype=np.float32) / 8,
                 b2=np.zeros(32, np.float32),
                 W2r=rng.standard_normal((64, 32), dtype=np.float32) / 8)
    print(out.shape, out.dtype)



# revision 7
# speedup vs baseline: 4.6401x; 4.6401x over previous
"""GraphSAGE 2-layer kernel for 8 Trainium2 NeuronCores (SPMD).

Strategy (v2):
- Nodes sorted by in-degree, padded to NPOS = 8*128*NBLK positions, dealt
  round-robin to cores in 128-lane blocks (run j = 8 cores' block j, all with
  near-equal degree). One reserved always-zero lane per core (last block,
  lane 127).
- Layer-1 aggregation: host expands x[src] into the canonical slot layout
  [128 lanes x cols x 64] bf16 (edges of dst lane p in column k); the device
  segment-sum is a plain PSUM accumulation of identity matmuls. Dense part
  per block: Y = [mean | x] @ [Wl; Wr] via one matmul with lhsT =
  [mean^T; x^T]; L2-normalize via ACT square+accum, sqrt, DVE reciprocal,
  fused scale+relu. h written as fp16 into a 256B/row layout.
- h (fp16, [*, 128] rows: cols 0:64 = h, 64:128 don't-care) is AllGathered.
- Layer-2 aggregation: edges partitioned by destination core; tokens sorted
  by (group g, window w, half h) and padded to 128 multiples per
  (g, half, w) subcell UNIFORMLY across cores (max over cores). One
  dma_gather per (g, w) fetches h[src] rows (256B each, int16 indices into
  one of 4 windows). Per 128-token chunk, a selection matrix T
  [128 tok, 512] is built ON DEVICE by DVE: T[t, col_t] = 1/deg(dst_t)
  (iota==colidx)*tval, and one PE matmul accumulates
  meanT[f, (b%4)*128+p] += h[tok, f] * T[tok, col] into a [64, 512] PSUM
  half-group tile -- mean arrives transposed with the 1/deg fold-in, no
  per-node padding, no scale or transpose steps.
- Dense part layer 2: actsT[0:64] <- meanT psum, actsT[64:128] <- PE
  transpose of own h rows; one matmul with [W2l; W2r], L2-normalize.
"""
import numpy as np
import ml_dtypes

import concourse.bass as bass
import concourse.bacc as bacc
import concourse.tile as tile
from concourse import mybir
from concourse import bass_utils

NCORES = 8
LANES = 128
BPG = 8           # blocks per group (psum accumulation granularity: 2 halves)
L1_CHUNK_COLS = 96
F_IN, F_HID, F_OUT = 64, 64, 32
BF16 = ml_dtypes.bfloat16
FP16 = np.float16
HROW = 128        # fp16 elements per h row (256B; cols 64:128 don't-care)


def _wrap_idx(flat_idx):
    """flat [n] int16 -> [128, n/16] wrapped in 16 partitions, replicated x8."""
    n = flat_idx.shape[0]
    arr = flat_idx.reshape(n // 16, 16).T
    return np.tile(arr, (8, 1)).astype(np.int16)


def _preprocess(x, edge_index, N):
    src = np.asarray(edge_index[0], dtype=np.int64)
    dst = np.asarray(edge_index[1], dtype=np.int64)
    E = src.shape[0]

    nblk = int(np.ceil((N + NCORES) / (NCORES * LANES)))
    npos = NCORES * LANES * nblk
    npc = LANES * nblk
    winrows = 2 * npc
    nwin = NCORES // 2
    ngrp = int(np.ceil(nblk / BPG))
    nb_g = [min(BPG, nblk - g * BPG) for g in range(ngrp)]

    deg = np.bincount(dst, minlength=N).astype(np.int64)

    # position space: all (run j, core c, lane p); reserved = (nblk-1, c, 127)
    res_pos = (nblk - 1) * NCORES * LANES + np.arange(NCORES) * LANES + (LANES - 1)
    avail = np.ones(npos, dtype=bool)
    avail[res_pos] = False
    avail_pos = np.flatnonzero(avail)
    nfill = npos - NCORES - N
    order = np.argsort(deg, kind="stable")
    pos2node = np.full(npos, -1, dtype=np.int64)
    pos2node[avail_pos[nfill:]] = order

    ii = np.arange(npos)
    pos_c = (ii % (NCORES * LANES)) // LANES
    pos_j = ii // (NCORES * LANES)
    pos_p = ii % LANES
    pos_row = pos_c * npc + pos_j * 128 + pos_p
    node2row = np.empty(N, dtype=np.int64)
    real = pos2node >= 0
    node2row[pos2node[real]] = pos_row[real]

    # per-run degree maxima -> L1 schedule
    degpos = np.where(real, deg[np.clip(pos2node, 0, None)], 0)
    run_deg = degpos.reshape(nblk, NCORES * LANES).max(axis=1)
    d1_g = [max(1, int(run_deg[g * BPG:g * BPG + nb_g[g]].max())) for g in range(ngrp)]

    # CSR by dst (for L1 slot expansion)
    eord = np.argsort(dst, kind="stable")
    s_by_dst = src[eord]
    indptr = np.zeros(N + 1, dtype=np.int64)
    indptr[1:] = np.cumsum(deg)

    xbf = np.asarray(x, dtype=np.float32).astype(BF16)
    xf = np.asarray(x, dtype=np.float32)

    # node id at (c, j, p)
    node_cjp = np.full((NCORES, nblk, LANES), -1, dtype=np.int64)
    node_cjp[pos_c[real], pos_j[real], pos_p[real]] = pos2node[real]

    deg_cjp = np.where(node_cjp >= 0, deg[np.clip(node_cjp, 0, None)], 0)
    ip_cjp = np.where(node_cjp >= 0, indptr[np.clip(node_cjp, 0, None)], 0)

    # ---- L1 slots expansion + schedule ----
    tot1 = sum(d1_g[g] * nb_g[g] for g in range(ngrp))
    slots1 = [np.zeros((128, tot1, F_IN), dtype=BF16) for _ in range(NCORES)]
    l1_sched = []  # per group: (col_offset, d1, nb)
    cofs = 0
    for g in range(ngrp):
        d1, nb = d1_g[g], nb_g[g]
        l1_sched.append((cofs, d1, nb))
        for b in range(nb):
            j = g * BPG + b
            for c in range(NCORES):
                db = deg_cjp[c, j]
                base = ip_cjp[c, j][:, None] + np.arange(d1)[None, :]
                valid = np.arange(d1)[None, :] < db[:, None]
                sidx = np.where(valid, s_by_dst[np.clip(base, 0, E - 1)], 0)
                vals = np.where(valid[:, :, None], xbf[sidx], BF16(0))
                slots1[c][:, cofs + b + np.arange(d1) * nb, :] = vals
        cofs += d1 * nb
    assert cofs == tot1

    # ---- L2 token streams (v2: T-matrix, no per-node padding) ----
    srow = node2row[src]
    drow = node2row[dst]
    dcore = drow // npc
    rr = drow % npc
    ej = rr // 128
    ep = rr % 128
    eg = ej // BPG
    eb = ej % BPG
    eh = eb // 4
    ecol = (eb % 4) * 128 + ep              # column in the [64, 512] half tile
    ew = srow // winrows
    erloc = srow % winrows
    etval = (1.0 / np.maximum(deg[dst], 1)).astype(np.float32)

    ncell = ngrp * nwin * 2
    cellid = (eg * nwin + ew) * 2 + eh       # stream order: g, w, h
    cnt = np.zeros((NCORES, ncell), dtype=np.int64)
    np.add.at(cnt, (dcore, cellid), 1)
    maxcnt = cnt.max(axis=0)
    ntok_cell = ((maxcnt + 127) // 128 * 128).astype(np.int64)
    nch_cell = ntok_cell // 128
    ofs_tok = np.zeros(ncell + 1, dtype=np.int64)
    ofs_tok[1:] = np.cumsum(ntok_cell)
    tot_tok = int(ofs_tok[-1])
    nch_total = tot_tok // 128
    ofs_ch = ofs_tok // 128

    idx2 = []
    colv = []
    tvalv = []
    for c in range(NCORES):
        m = dcore == c
        so = np.lexsort((ep[m], eb[m], cellid[m]))
        ck = cellid[m][so]
        first = np.searchsorted(ck, np.arange(ncell), side="left")
        pos_in_cell = np.arange(len(ck)) - first[ck]
        gpos = ofs_tok[ck] + pos_in_cell
        rl = np.zeros(tot_tok, dtype=np.int64)
        cv = np.zeros(tot_tok, dtype=np.float32)
        tv = np.zeros(tot_tok, dtype=np.float32)
        rl[gpos] = erloc[m][so]
        cv[gpos] = ecol[m][so]
        tv[gpos] = etval[m][so]
        idx2.append(_wrap_idx(rl.astype(np.int16)))
        colv.append(cv.reshape(nch_total, 128).T.astype(np.float32))
        tvalv.append(tv.reshape(nch_total, 128).T.astype(np.float32))

    # per-(g,w) gather extents + per-(g,h) chunk totals
    l2_gw = []   # [g][w] = (ntok, [(h, nch), ...])
    tot_gh = np.zeros((ngrp, 2), dtype=np.int64)
    maxtok_gw = 0
    maxch_gw = 0
    for g in range(ngrp):
        row = []
        for w in range(nwin):
            cl = (g * nwin + w) * 2
            ntok = int(ntok_cell[cl] + ntok_cell[cl + 1])
            hs = [(h, int(nch_cell[cl + h])) for h in range(2)
                  if nch_cell[cl + h] > 0]
            row.append((ntok, hs))
            maxtok_gw = max(maxtok_gw, ntok)
            maxch_gw = max(maxch_gw, ntok // 128)
            for h in range(2):
                tot_gh[g, h] += int(nch_cell[cl + h])
        l2_gw.append(row)

    # ---- dense inputs (L1) ----
    xT = np.zeros((NCORES, nblk, F_IN, 128), dtype=np.float32)
    invc = np.zeros((NCORES, 128, nblk), dtype=np.float32)
    for c in range(NCORES):
        nodes = node_cjp[c]
        ok = nodes >= 0
        xv = np.where(ok[:, :, None], xf[np.clip(nodes, 0, None)], 0.0)
        xT[c] = xv.transpose(0, 2, 1)
        invc[c] = np.where(ok, 1.0 / np.maximum(deg_cjp[c], 1), 0.0).T

    meta = dict(nblk=nblk, npos=npos, npc=npc, winrows=winrows, nwin=nwin,
                ngrp=ngrp, nb_g=nb_g, d1_g=d1_g, l1_sched=l1_sched, tot1=tot1,
                l2_gw=l2_gw, tot_gh=tot_gh, tot_tok=tot_tok,
                nch_total=nch_total, maxtok_gw=maxtok_gw, maxch_gw=maxch_gw,
                node2row=node2row)
    per_core = dict(slots1=[s.reshape(128, tot1 * F_IN) for s in slots1],
                    idx2=idx2, colv=colv, tvalv=tvalv, xT=xT, invc=invc)
    return meta, per_core


def _build(meta):
    nblk, npc, npos = meta["nblk"], meta["npc"], meta["npos"]
    winrows, nwin, ngrp = meta["winrows"], meta["nwin"], meta["ngrp"]
    nb_g, l1_sched, tot1 = meta["nb_g"], meta["l1_sched"], meta["tot1"]
    l2_gw, tot_gh = meta["l2_gw"], meta["tot_gh"]
    tot_tok, nch_total = meta["tot_tok"], meta["nch_total"]
    maxtok_gw, maxch_gw = meta["maxtok_gw"], meta["maxch_gw"]

    nc = bacc.Bacc("TRN2", target_bir_lowering=False, debug=False,
                   num_devices=NCORES)
    slots1 = nc.dram_tensor("slots1", [128, tot1 * F_IN], mybir.dt.bfloat16,
                            kind="ExternalInput")
    idx2 = nc.dram_tensor("idx2", [128, tot_tok // 16], mybir.dt.int16,
                          kind="ExternalInput")
    colv_d = nc.dram_tensor("colv", [128, nch_total], mybir.dt.float32,
                            kind="ExternalInput")
    tvalv_d = nc.dram_tensor("tvalv", [128, nch_total], mybir.dt.float32,
                             kind="ExternalInput")
    xT = nc.dram_tensor("xT", [nblk, F_IN, 128], mybir.dt.float32,
                        kind="ExternalInput")
    invc_d = nc.dram_tensor("invc", [128, nblk], mybir.dt.float32,
                            kind="ExternalInput")
    w1s = nc.dram_tensor("w1s", [128, F_HID], mybir.dt.float32,
                         kind="ExternalInput")
    w2s = nc.dram_tensor("w2s", [128, F_OUT], mybir.dt.float32,
                         kind="ExternalInput")
    b1t = nc.dram_tensor("b1t", [128, F_HID], mybir.dt.float32,
                         kind="ExternalInput")
    b2t = nc.dram_tensor("b2t", [128, F_OUT], mybir.dt.float32,
                         kind="ExternalInput")
    identf = nc.dram_tensor("identf", [128, 128], mybir.dt.float32,
                            kind="ExternalInput")
    identb = nc.dram_tensor("identb", [128, 128], mybir.dt.bfloat16,
                            kind="ExternalInput")
    identh = nc.dram_tensor("identh", [128, 128], mybir.dt.float16,
                            kind="ExternalInput")
    iota_d = nc.dram_tensor("iota", [128, 512], mybir.dt.float16,
                            kind="ExternalInput")
    out_d = nc.dram_tensor("out", [npc, F_OUT], mybir.dt.float32,
                           kind="ExternalOutput")

    with tile.TileContext(nc) as tc:
        with (
            tc.tile_pool(name="const", bufs=1) as cp,
            tc.tile_pool(name="slots", bufs=3) as sp,
            tc.tile_pool(name="gath", bufs=3) as gp,
            tc.tile_pool(name="idxp", bufs=3) as ixp,
            tc.tile_pool(name="tmat", bufs=4) as tp,
            tc.tile_pool(name="blk", bufs=3) as bp,
            tc.tile_pool(name="psT", bufs=2, space="PSUM") as psT,
            tc.tile_pool(name="psD", bufs=2, space="PSUM") as psD,
            tc.tile_pool(name="dram", bufs=1, space="DRAM") as dp,
        ):
            idf = cp.tile([128, 128], mybir.dt.float32, tag="idf")
            nc.sync.dma_start(idf[:], identf[:])
            idb = cp.tile([128, 128], mybir.dt.bfloat16, tag="idb")
            nc.sync.dma_start(idb[:], identb[:])
            idh = cp.tile([128, 128], mybir.dt.float16, tag="idh")
            nc.sync.dma_start(idh[:], identh[:])
            iot = cp.tile([128, 512], mybir.dt.float16, tag="iot")
            nc.sync.dma_start(iot[:], iota_d[:])
            w1 = cp.tile([128, F_HID], mybir.dt.float32, tag="w1")
            nc.sync.dma_start(w1[:], w1s[:])
            w2 = cp.tile([128, F_OUT], mybir.dt.float32, tag="w2")
            nc.sync.dma_start(w2[:], w2s[:])
            bt1 = cp.tile([128, F_HID], mybir.dt.float32, tag="bt1")
            nc.sync.dma_start(bt1[:], b1t[:])
            bt2 = cp.tile([128, F_OUT], mybir.dt.float32, tag="bt2")
            nc.sync.dma_start(bt2[:], b2t[:])
            icn = cp.tile([128, nblk], mybir.dt.float32, tag="icn")
            nc.sync.dma_start(icn[:], invc_d[:])
            epst = cp.tile([128, 1], mybir.dt.float32, tag="epst")
            nc.vector.memset(epst[:], 1e-24)

            hshard = dp.tile([npc, HROW], mybir.dt.float16)
            hgat = dp.tile([npos, HROW], mybir.dt.float16)

            def norm_out(y, fdim, relu, outdst, odt):
                """L2-normalize rows of y [128, fdim] (+opt relu), write out."""
                sq = bp.tile([128, F_HID], mybir.dt.float32, tag="sq")
                ss = bp.tile([128, 1], mybir.dt.float32, tag="ss")
                nc.scalar.activation(out=sq[:, :fdim], in_=y[:],
                                     func=mybir.ActivationFunctionType.Square,
                                     accum_out=ss[:])
                s = bp.tile([128, 1], mybir.dt.float32, tag="s")
                nc.scalar.activation(out=s[:], in_=ss[:],
                                     func=mybir.ActivationFunctionType.Sqrt,
                                     bias=epst[:])
                rv = bp.tile([128, 1], mybir.dt.float32, tag="rv")
                nc.vector.reciprocal(rv[:], s[:])
                o = bp.tile([128, F_HID], odt, tag="o")
                if relu:
                    nc.vector.tensor_scalar(out=o[:, :fdim], in0=y[:],
                                            scalar1=rv[:], scalar2=0.0,
                                            op0=mybir.AluOpType.mult,
                                            op1=mybir.AluOpType.max)
                else:
                    nc.vector.tensor_scalar_mul(o[:, :fdim], y[:], rv[:])
                nc.sync.dma_start(outdst, o[:, :fdim])

            # ---------------- layer 1 ----------------
            psA_ctx = tc.tile_pool(name="psA", bufs=2, space="PSUM")
            psA = psA_ctx.__enter__()
            for g in range(ngrp):
                cofs, d1, nb = l1_sched[g]
                pa = psA.tile([128, 512], mybir.dt.float32, space="PSUM", tag="pa")
                k0 = 0
                first = True
                while k0 < d1:
                    nk = min(max(1, L1_CHUNK_COLS // nb), d1 - k0)
                    ncols = nk * nb
                    st = sp.tile([128, L1_CHUNK_COLS * F_IN], mybir.dt.bfloat16,
                                 tag="st")
                    nc.sync.dma_start(
                        st[:, :ncols * F_IN],
                        slots1[:, (cofs + k0 * nb) * F_IN:
                               (cofs + (k0 + nk) * nb) * F_IN])
                    for k in range(nk):
                        last = (k0 + k == d1 - 1)
                        nc.tensor.matmul(
                            out=pa[:, :nb * F_IN],
                            lhsT=idb[:],
                            rhs=st[:, k * nb * F_IN:(k + 1) * nb * F_IN],
                            start=first, stop=last)
                        first = False
                    k0 += nk
                for b in range(nb):
                    j = g * BPG + b
                    mean = bp.tile([128, F_IN], mybir.dt.float32, tag="mean")
                    nc.vector.tensor_scalar_mul(
                        mean[:], pa[:, b * F_IN:(b + 1) * F_IN],
                        icn[:, j:j + 1])
                    mT = psT.tile([64, 128], mybir.dt.float32, space="PSUM",
                                  tag="mT")
                    nc.tensor.transpose(out=mT[:], in_=mean[:], identity=idf[:])
                    actsT = bp.tile([128, 128], mybir.dt.float32, tag="actsT")
                    nc.vector.tensor_copy(actsT[0:64, :], mT[:])
                    nc.sync.dma_start(actsT[64:128, :], xT[j, :, :])
                    pd = psD.tile([128, F_HID], mybir.dt.float32, space="PSUM",
                                  tag="pd")
                    nc.tensor.matmul(out=pd[:], lhsT=actsT[:], rhs=w1[:],
                                     start=True, stop=True)
                    y = bp.tile([128, F_HID], mybir.dt.float32, tag="y")
                    nc.vector.tensor_tensor(out=y[:], in0=pd[:], in1=bt1[:],
                                            op=mybir.AluOpType.add)
                    norm_out(y, F_HID, True,
                             hshard[j * 128:(j + 1) * 128, 0:F_HID],
                             mybir.dt.float16)

            psA_ctx.__exit__(None, None, None)

            # ---------------- all-gather ----------------
            nc.gpsimd.collective_compute(
                "AllGather", mybir.AluOpType.bypass,
                replica_groups=[list(range(NCORES))],
                ins=[hshard[:]], outs=[hgat[:]])

            # ---------------- layer 2 ----------------
            psM_ctx = tc.tile_pool(name="psM", bufs=2, space="PSUM")
            psM = psM_ctx.__enter__()
            iofs = 0
            chofs = 0
            seen = np.zeros((ngrp, 2), dtype=np.int64)
            for g in range(ngrp):
                nb = nb_g[g]
                psml = psM.tile([64, 512], mybir.dt.float32, space="PSUM",
                                tag="psml")
                psmh = psM.tile([64, 512], mybir.dt.float32, space="PSUM",
                                tag="psmh")
                psm = [psml, psmh]
                for w in range(nwin):
                    ntok, hs = l2_gw[g][w]
                    if ntok == 0:
                        continue
                    nch = ntok // 128
                    it = ixp.tile([128, maxtok_gw // 16], mybir.dt.int16,
                                  tag="it")
                    nc.sync.dma_start(it[:, :ntok // 16],
                                      idx2[:, iofs:iofs + ntok // 16])
                    gt = gp.tile([128, maxch_gw * HROW], mybir.dt.float16,
                                 tag="gt")
                    gt3 = gt[:, :nch * HROW].rearrange("p (c f) -> p c f",
                                                       c=nch)
                    nc.gpsimd.dma_gather(
                        out_ap=gt3,
                        in_ap=hgat[w * winrows:(w + 1) * winrows, :],
                        idxs_ap=it[:, :ntok // 16],
                        num_idxs=ntok,
                        num_idxs_reg=ntok,
                        elem_size=HROW,
                        single_packet=False)
                    cv = ixp.tile([128, maxch_gw], mybir.dt.float32, tag="cv")
                    nc.sync.dma_start(cv[:, :nch], colv_d[:, chofs:chofs + nch])
                    tv = ixp.tile([128, maxch_gw], mybir.dt.float32, tag="tv")
                    nc.sync.dma_start(tv[:, :nch],
                                      tvalv_d[:, chofs:chofs + nch])
                    k = 0
                    for h, nchh in hs:
                        for _ in range(nchh):
                            tt = tp.tile([128, 512], mybir.dt.float16,
                                         tag="tt")
                            nc.vector.tensor_scalar(
                                out=tt[:], in0=iot[:],
                                scalar1=cv[:, k:k + 1],
                                scalar2=tv[:, k:k + 1],
                                op0=mybir.AluOpType.is_equal,
                                op1=mybir.AluOpType.mult)
                            st_ = seen[g, h] == 0
                            seen[g, h] += 1
                            sp_ = seen[g, h] == tot_gh[g, h]
                            nc.tensor.matmul(
                                out=psm[h][:],
                                lhsT=gt3[:, k, 0:F_HID],
                                rhs=tt[:],
                                start=bool(st_), stop=bool(sp_))
                            k += 1
                    iofs += ntok // 16
                    chofs += nch
                for b in range(nb):
                    j = g * BPG + b
                    h = b // 4
                    actsT = bp.tile([128, 128], mybir.dt.float32, tag="actsT")
                    if tot_gh[g, h] == 0:
                        nc.vector.memset(actsT[0:64, :], 0.0)
                    else:
                        nc.vector.tensor_copy(
                            actsT[0:64, :],
                            psm[h][:, (b % 4) * 128:(b % 4 + 1) * 128])
                    hr = bp.tile([128, F_HID], mybir.dt.float16, tag="hr")
                    nc.sync.dma_start(hr[:],
                                      hshard[j * 128:(j + 1) * 128, 0:F_HID])
                    hT = psT.tile([64, 128], mybir.dt.float16, space="PSUM",
                                  tag="mT")
                    nc.tensor.transpose(out=hT[:], in_=hr[:], identity=idh[:])
                    nc.scalar.copy(actsT[64:128, :], hT[:])
                    pd = psD.tile([128, F_HID], mybir.dt.float32, space="PSUM",
                                  tag="pd")
                    nc.tensor.matmul(out=pd[:, :F_OUT], lhsT=actsT[:],
                                     rhs=w2[:], start=True, stop=True)
                    y = bp.tile([128, F_HID], mybir.dt.float32, tag="y")
                    nc.vector.tensor_tensor(out=y[:, :F_OUT],
                                            in0=pd[:, :F_OUT], in1=bt2[:],
                                            op=mybir.AluOpType.add)
                    norm_out(y[:, :F_OUT], F_OUT, False,
                             out_d[j * 128:(j + 1) * 128, :],
                             mybir.dt.float32)
            psM_ctx.__exit__(None, None, None)
    nc.compile()
    return nc


def make_in_maps(meta, per_core, W1l, b1, W1r, W2l, b2, W2r):
    w1s = np.concatenate([np.asarray(W1l, np.float32),
                          np.asarray(W1r, np.float32)], axis=0)
    w2s = np.concatenate([np.asarray(W2l, np.float32),
                          np.asarray(W2r, np.float32)], axis=0)
    b1t = np.tile(np.asarray(b1, np.float32)[None, :], (128, 1))
    b2t = np.tile(np.asarray(b2, np.float32)[None, :], (128, 1))
    identf = np.eye(128, dtype=np.float32)
    iota = np.tile(np.arange(512, dtype=FP16)[None, :], (128, 1))
    in_maps = []
    for c in range(NCORES):
        in_maps.append(dict(
            slots1=per_core["slots1"][c],
            idx2=per_core["idx2"][c],
            colv=per_core["colv"][c],
            tvalv=per_core["tvalv"][c],
            xT=per_core["xT"][c],
            invc=per_core["invc"][c],
            w1s=w1s, w2s=w2s, b1t=b1t, b2t=b2t,
            identf=identf, identb=identf.astype(BF16),
            identh=identf.astype(FP16), iota=iota,
        ))
    return in_maps


def kernel(x, edge_index, W1l, b1, W1r, W2l, b2, W2r):
    x = np.asarray(x, dtype=np.float32)
    N = x.shape[0]
    meta, per_core = _preprocess(x, edge_index, N)
    nc = _build(meta)
    in_maps = make_in_maps(meta, per_core, W1l, b1, W1r, W2l, b2, W2r)
    res = bass_utils.run_bass_kernel_spmd(nc, in_maps,
                                          core_ids=list(range(NCORES)))
    outs = np.concatenate([res.results[c]["out"] for c in range(NCORES)],
                          axis=0)
    full = outs[meta["node2row"]]
    return full.astype(np.float32)


if __name__ == "__main__":
    rng = np.random.default_rng(0)
    N, E = 100000, 1000000
    x = rng.standard_normal((N, 64), dtype=np.float32)
    ei = rng.integers(0, N, size=(2, E)).astype(np.int64)
    out = kernel(x=x, edge_index=ei,
                 W1l=rng.standard_normal((64, 64), dtype=np.float32) / 8,
                 b1=np.zeros(64, np.float32),
                 W1r=rng.standard_normal((64, 64), dtype=np.float32) / 8,
                 W2l=rng.standard_normal((64, 32), dtype=np.float32) / 8,
                 b2=np.zeros(32, np.float32),
                 W2r=rng.standard_normal((64, 32), dtype=np.float32) / 8)
    print(out.shape, out.dtype)
